# revision 18
# baseline (speedup 1.0000x reference)
"""Trainium2 Bass kernel for a PointNet-style neighborhood encoder.

Computation (matches the reference nn.Module):
    h = relu(relu(relu(points @ W0 + b0) @ W1 + b1) @ W2 + b2)   # [N,3] -> [N,128]
    pooled = segment_max(h, cluster)                             # [C,128], 32 pts/cluster
    out = relu(relu(pooled @ G0 + g0) @ G1 + g1)                 # [C,256]

Sharding: data-parallel over points across 8 NeuronCores (cluster
boundaries are shard-aligned because clusters are contiguous, 32
points each). Weights are replicated. No collectives; the host
scatters inputs and gathers per-core outputs.

Device strategy (per core, n = 262144 points = 65536 quad-columns):
  - Host packs points feature-major, 4 points per 128-partition column
    ("quads"): pts4[3a+f, q] = points[4q+a, f], so layer 0 is a single
    block-diagonal matmul (K=12, M=128) producing h0 for 4 points/col.
  - Layer 1 uses two permuted block-diagonal stationaries W1A/W1B
    (K=128, M=128) producing h1 with 2 points per column.
  - Layer 2 uses W2 duplicated on both partition halves; 4 sub-matmuls
    (K=64, M=128) with rhs partition slices map to distinct PE row
    groups, producing z = W2^T h1 (bias/relu deferred) in PSUM.
  - segment_max: relu is monotone and b2 is constant per feature, so
    pooled = relu(max_p(z) + b2). max over (4 tensors x 8 quads) is ONE
    VectorE tensor_reduce(axis=XY) straight out of PSUM per sub-chunk.
  - ScalarE (ACT) does every relu+bias PSUM->SBUF evacuation; VectorE
    only does the pooling reduces. bf16 activations everywhere
    (PSUM stays f32 as the HW requires).
  - Global MLP on pooled [128, 8192] per core; output is written
    feature-major [256, 8192] bf16 and transposed/upcast on the host.
"""

import numpy as np

# ---- problem geometry (hardcoded per contract) ----
N = 2097152          # total points
C = 65536            # clusters
PTS = 32             # points per cluster
NCORES = 8
NPC = N // NCORES    # points per core = 262144
N4C = NPC // 4       # quad-columns per core = 65536
CPC = C // NCORES    # clusters per core = 8192

BIG = 1024           # quad-columns per big-chunk
SUB = 256            # quad-columns per L2/pool sub-chunk
NCHUNK = N4C // BIG  # 64
NSUB = BIG // SUB    # 4

_CACHE = {}


def _bf16():
    import ml_dtypes
    return ml_dtypes.bfloat16


def _build_module(n4c: int):
    """Build the Bass module (SPMD program, same for all cores)."""
    import concourse.bass as bass
    import concourse.bacc as bacc
    import concourse.tile as tile
    from concourse import mybir

    BF = mybir.dt.bfloat16
    F32 = mybir.dt.float32
    RELU = mybir.ActivationFunctionType.Relu
    MAX = mybir.AluOpType.max
    ADD = mybir.AluOpType.add
    AXX = mybir.AxisListType.X

    nchunk = n4c // BIG
    cpc = n4c // 8          # clusters per core for this size

    nc = bacc.Bacc()

    # ---- DRAM I/O ----
    # weights batched into two tensors so the prologue is 3 parallel DMAs
    # (13 small serialized DMAs used to block the Pool queue for ~6.5us)
    pts4 = nc.dram_tensor("pts4", [12, n4c], BF, kind="ExternalInput")
    w0q = nc.dram_tensor("w0q", [12, 128], BF, kind="ExternalInput")
    wbig = nc.dram_tensor("wbig", [128, 768], BF, kind="ExternalInput")
    bbig = nc.dram_tensor("bbig", [128, 6], F32, kind="ExternalInput")
    outt = nc.dram_tensor("outt", [256, cpc], BF, kind="ExternalOutput")

    from contextlib import ExitStack
    with tile.TileContext(nc) as tc, ExitStack() as ctx:
        singles = ctx.enter_context(tc.tile_pool(name="singles", bufs=1))
        ppts = ctx.enter_context(tc.tile_pool(name="ppts", bufs=3))
        ph0s = ctx.enter_context(tc.tile_pool(name="ph0s", bufs=2))
        ph1s = ctx.enter_context(tc.tile_pool(name="ph1s", bufs=3))
        psA = ctx.enter_context(tc.tile_pool(name="psA", bufs=3))
        psB = ctx.enter_context(tc.tile_pool(name="psB", bufs=2))
        psC = ctx.enter_context(tc.tile_pool(name="psC", bufs=2))
        psum_h0 = ctx.enter_context(tc.tile_pool(name="psum_h0", bufs=1, space="PSUM"))
        psum_h1 = ctx.enter_context(tc.tile_pool(name="psum_h1", bufs=2, space="PSUM"))
        psum_z = ctx.enter_context(tc.tile_pool(name="psum_z", bufs=3, space="PSUM"))

        # ---- load constants: 3 batched DMAs on otherwise-idle queues ----
        wbig_s = singles.tile([128, 768], BF)
        nc.gpsimd.dma_start(out=wbig_s[:], in_=wbig[:])
        w0q_s = singles.tile([12, 128], BF)
        nc.scalar.dma_start(out=w0q_s[:], in_=w0q[:])
        bbig_s = singles.tile([128, 6], F32)
        nc.scalar.dma_start(out=bbig_s[:], in_=bbig[:])
        w1a_s = wbig_s[:, 0:128]
        w1b_s = wbig_s[:, 128:256]
        w2d_s = wbig_s[:, 256:384]
        g0w_s = wbig_s[:, 384:512]
        g1lo_s = wbig_s[:, 512:640]
        g1hi_s = wbig_s[:, 640:768]
        b0q_s = bbig_s[:, 0:1]
        b1d_s = bbig_s[:, 1:2]
        b2v_s = bbig_s[:, 2:3]
        g0v_s = bbig_s[:, 3:4]
        g1l_s = bbig_s[:, 4:5]
        g1h_s = bbig_s[:, 5:6]

        # pooled max(z) accumulator for the whole core
        pooled = singles.tile([128, cpc], BF)

        # ---- main loop; L2/pool of chunk i-2 runs during chunk i ----
        # The 2-chunk shift lets the L1 evacuations be two single
        # [128,1024] ACT instructions (cheapest form) and still complete
        # well before their L2 consumers.
        # Per chunk (1024 quad-cols = 4096 points) steady-state budget:
        #   PE   L0 2x512 + L1 4x512 + L2 16x256 + G mms      ~3150 ns
        #   ACT  2x h0 evac + 2x [128,1024] h1 evac           ~3300 ns
        #   DVE  4x A-TT (PSUM, the only engine that can) + G ~3130 ns
        #   Pool B/C tree stages + g0in (SBUF-only by HW rule) ~1710 ns
        # Max-pool tree (4096 z cols -> 128 pooled, max over 32 pts):
        #   A (per sub, DVE):   TT max(tileA, tileB) PSUM->bf16 SBUF
        #   B (per pair, Pool): TT max of a/b halves
        #   C1/C2/C3 (Pool):    TT chain folding quads 8->4->2->1
        # PSUM (8 banks): h0 ring 1x[128,512], h1 ring 2x[128,1024] (also
        # hosts the [128,512] G-MLP matmul tiles), z ring 3x[128,512].
        cur_sB = {}

        def emit_l2_pair(i, jpair, h1as, h1bs):
            """L2 matmuls for subs (2*jpair, 2*jpair+1) of chunk i: per sub
            two [128,512] z tiles (tileA = a0|b0, tileB = a1|b1), then the
            A-stage TT on DVE merges them into sAp."""
            if jpair == 0:
                sBt = psB.tile([128, BIG], BF, tag="sB")
                cur_sB[i] = sBt
            else:
                sBt = cur_sB[i]
            sAp = psA.tile([128, 4 * SUB], BF, tag="sA")
            for idx, j in ((0, 2 * jpair), (1, 2 * jpair + 1)):
                s0 = j * SUB
                zpa = psum_z.tile([128, 2 * SUB], F32, tag="zp")
                zpb = psum_z.tile([128, 2 * SUB], F32, tag="zp")
                nc.tensor.matmul(zpa[:, 0:SUB],
                                 w2d_s[0:64, :], h1as[0:64, s0:s0 + SUB])
                nc.tensor.matmul(zpa[:, SUB:2 * SUB],
                                 w2d_s[0:64, :], h1bs[0:64, s0:s0 + SUB])
                nc.tensor.matmul(zpb[:, 0:SUB],
                                 w2d_s[64:128, :], h1as[64:128, s0:s0 + SUB])
                nc.tensor.matmul(zpb[:, SUB:2 * SUB],
                                 w2d_s[64:128, :], h1bs[64:128, s0:s0 + SUB])
                # A: merge point-groups {a0,a1} and {b0,b1} (DVE; Pool
                # cannot read PSUM on real HW)
                nc.vector.tensor_tensor(
                    out=sAp[:, idx * 512:(idx + 1) * 512],
                    in0=zpa[:], in1=zpb[:], op=MAX)
            # B: merge a/b -> (sub, cluster, quad) layout, on Pool
            sAv = sAp.rearrange("p (s t x) -> p s t x", s=2, t=2)
            nc.gpsimd.tensor_tensor(
                out=sBt[:, jpair * 512:(jpair + 1) * 512],
                in0=sAv[:, :, 0:1, :], in1=sAv[:, :, 1:2, :], op=MAX)

        def emit_pool_c(i):
            # C: fold the 8 quads per cluster with a TT chain on Pool
            sBt = cur_sB.pop(i)
            base = i * (BIG // 8)
            sC1 = psC.tile([128, 512], BF, tag="sC1")
            sBv = sBt.rearrange("p (c q) -> p c q", q=8)
            nc.gpsimd.tensor_tensor(out=sC1[:], in0=sBv[:, :, 0:4],
                                    in1=sBv[:, :, 4:8], op=MAX)
            sC2 = psC.tile([128, 256], BF, tag="sC2")
            sC1v = sC1.rearrange("p (c q) -> p c q", q=4)
            nc.gpsimd.tensor_tensor(out=sC2[:], in0=sC1v[:, :, 0:2],
                                    in1=sC1v[:, :, 2:4], op=MAX)
            sC2v = sC2.rearrange("p (c q) -> p c q", q=2)
            nc.gpsimd.tensor_tensor(out=pooled[:, base:base + BIG // 8],
                                    in0=sC2v[:, :, 0:1], in1=sC2v[:, :, 1:2],
                                    op=MAX)

        g0in = singles.tile([128, cpc], BF)
        g1in = singles.tile([128, cpc], BF)
        goutL = singles.tile([128, cpc], BF)
        goutH = singles.tile([128, cpc], BF)

        # global-MLP tasks at 512-cluster granularity (one G matmul tile
        # [128,512] per task, allocated from the h1 PSUM ring; at most one
        # task per chunk so the ring rotation stays safe)
        def g_task_g0(k):
            sl = slice(k * 512, (k + 1) * 512)
            # relu(x + b) as (x add bias) max 0: g0in on Pool (SBUF only)
            nc.gpsimd.tensor_scalar(out=g0in[:, sl], in0=pooled[:, sl],
                                    scalar1=b2v_s[:], scalar2=0.0,
                                    op0=ADD, op1=MAX)
            gp = psum_h1.tile([128, 512], F32, tag="h1p")
            nc.tensor.matmul(gp[:], g0w_s[:], g0in[:, sl])
            nc.vector.tensor_scalar(out=g1in[:, sl], in0=gp[:],
                                    scalar1=g0v_s[:], scalar2=0.0,
                                    op0=ADD, op1=MAX)

        def g_task_lo(k):
            sl = slice(k * 512, (k + 1) * 512)
            gpl = psum_h1.tile([128, 512], F32, tag="h1p")
            nc.tensor.matmul(gpl[:], g1lo_s[:], g1in[:, sl])
            nc.vector.tensor_scalar(out=goutL[:, sl], in0=gpl[:],
                                    scalar1=g1l_s[:], scalar2=0.0,
                                    op0=ADD, op1=MAX)
            nc.sync.dma_start(out=outt[0:128, sl], in_=goutL[:, sl])

        def g_task_hi(k):
            sl = slice(k * 512, (k + 1) * 512)
            gph = psum_h1.tile([128, 512], F32, tag="h1p")
            nc.tensor.matmul(gph[:], g1hi_s[:], g1in[:, sl])
            nc.vector.tensor_scalar(out=goutH[:, sl], in0=gph[:],
                                    scalar1=g1h_s[:], scalar2=0.0,
                                    op0=ADD, op1=MAX)
            nc.sync.dma_start(out=outt[128:256, sl], in_=goutH[:, sl])

        g_tasks = []

        def pop_g_task():
            if g_tasks:
                fn, k = g_tasks.pop(0)
                fn(k)

        # Software-pipelined loop. Iteration `it` emits, interleaved:
        #   - L0 of chunk it        (so the h0 evacs precede the h1 evacs
        #                            of chunk it-1 in the ACT queue and no
        #                            ACT->PE->ACT serialization cycle forms)
        #   - L1 of chunk it-1
        #   - L2 + pooling of chunk it-2
        # L2 sub matmuls interleave with the L0/L1 matmuls so the PE covers
        # the A-stage latencies on the 3-slot z ring.
        hist = {}      # chunk -> (h1as, h1bs) awaiting their L2 stage
        h0_cur = None  # h0s of the chunk whose L1 stage runs next

        def emit_sub(j, h1as, h1bs, sAp):
            """One L2 sub (4 matmuls into two [128,512] z tiles) + A-TT."""
            s0 = j * SUB
            zpa = psum_z.tile([128, 2 * SUB], F32, tag="zp")
            zpb = psum_z.tile([128, 2 * SUB], F32, tag="zp")
            nc.tensor.matmul(zpa[:, 0:SUB],
                             w2d_s[0:64, :], h1as[0:64, s0:s0 + SUB])
            nc.tensor.matmul(zpa[:, SUB:2 * SUB],
                             w2d_s[0:64, :], h1bs[0:64, s0:s0 + SUB])
            nc.tensor.matmul(zpb[:, 0:SUB],
                             w2d_s[64:128, :], h1as[64:128, s0:s0 + SUB])
            nc.tensor.matmul(zpb[:, SUB:2 * SUB],
                             w2d_s[64:128, :], h1bs[64:128, s0:s0 + SUB])
            nc.vector.tensor_tensor(
                out=sAp[:, (j % 2) * 512:(j % 2 + 1) * 512],
                in0=zpa[:], in1=zpb[:], op=MAX)

        def emit_b(jpair, sAp, sBt):
            sAv = sAp.rearrange("p (s t x) -> p s t x", s=2, t=2)
            nc.gpsimd.tensor_tensor(
                out=sBt[:, jpair * 512:(jpair + 1) * 512],
                in0=sAv[:, :, 0:1, :], in1=sAv[:, :, 1:2, :], op=MAX)

        for it in range(nchunk + 1):
            do_l0 = it < nchunk
            ip = it - 2          # chunk whose L2+pool stage runs now
            prev_ready = ip >= 0

            if prev_ready:
                pm = hist[ip]
                sBt = psB.tile([128, BIG], BF, tag="sB")
                cur_sB[ip] = sBt
                sA0 = psA.tile([128, 4 * SUB], BF, tag="sA")
                sA1 = psA.tile([128, 4 * SUB], BF, tag="sA")

            if do_l0:
                c0 = it * BIG
                pts_t = ppts.tile([12, BIG], BF, tag="pts")
                nc.sync.dma_start(out=pts_t[:], in_=pts4[:, c0:c0 + BIG])
                h0s_n = ph0s.tile([128, BIG], BF, tag="h0s")
                h0qa = psum_h0.tile([128, 512], F32, tag="h0q")
                nc.tensor.matmul(h0qa[:], w0q_s[:], pts_t[:, 0:512])
                nc.scalar.activation(h0s_n[:, 0:512], h0qa[:], RELU,
                                     bias=b0q_s[:])

            if prev_ready:
                emit_sub(0, pm[0], pm[1], sA0)

            if do_l0:
                h0qb = psum_h0.tile([128, 512], F32, tag="h0q")
                nc.tensor.matmul(h0qb[:], w0q_s[:], pts_t[:, 512:1024])
                nc.scalar.activation(h0s_n[:, 512:1024], h0qb[:], RELU,
                                     bias=b0q_s[:])
            if it == 0:
                h0_cur = h0s_n
                continue
            h0s = h0_cur

            # L1A of chunk it-1, interleaved with L2 subs of chunk it-2
            h1as = ph1s.tile([128, BIG], BF, tag="h1as")
            h1bs = ph1s.tile([128, BIG], BF, tag="h1bs")
            h1pa = psum_h1.tile([128, BIG], F32, tag="h1p")
            nc.tensor.matmul(h1pa[:, 0:512], w1a_s[:], h0s[:, 0:512])
            nc.tensor.matmul(h1pa[:, 512:1024], w1a_s[:], h0s[:, 512:1024])
            nc.scalar.activation(h1as[:], h1pa[:], RELU, bias=b1d_s[:])

            if prev_ready:
                emit_sub(1, pm[0], pm[1], sA0)
                emit_b(0, sA0, sBt)

            # L1B of chunk it-1
            h1pb = psum_h1.tile([128, BIG], F32, tag="h1p")
            nc.tensor.matmul(h1pb[:, 0:512], w1b_s[:], h0s[:, 0:512])
            nc.tensor.matmul(h1pb[:, 512:1024], w1b_s[:], h0s[:, 512:1024])
            nc.scalar.activation(h1bs[:], h1pb[:], RELU, bias=b1d_s[:])

            if prev_ready:
                emit_sub(2, pm[0], pm[1], sA1)
            pop_g_task()
            if prev_ready:
                emit_sub(3, pm[0], pm[1], sA1)
                emit_b(1, sA1, sBt)
                emit_pool_c(ip)
                del hist[ip]

            hist[it - 1] = (h1as, h1bs)
            h0_cur = h0s_n if do_l0 else None

            # global-MLP work: chunk c pools during iteration c+2, so
            # block k (chunks 4k..4k+3) is ready from iteration 4k+5 on.
            if it >= 5 and (it - 5) % 4 == 0:
                k = (it - 5) // 4
                g_tasks.extend([(g_task_g0, k), (g_task_lo, k),
                                (g_task_hi, k)])

        # epilogue: the last chunk's L2+pool, then leftover G tasks
        L = nchunk - 1
        pm = hist[L]
        sBt = psB.tile([128, BIG], BF, tag="sB")
        cur_sB[L] = sBt
        sA0 = psA.tile([128, 4 * SUB], BF, tag="sA")
        sA1 = psA.tile([128, 4 * SUB], BF, tag="sA")
        emit_sub(0, pm[0], pm[1], sA0)
        pop_g_task()
        emit_sub(1, pm[0], pm[1], sA0)
        emit_b(0, sA0, sBt)
        pop_g_task()
        emit_sub(2, pm[0], pm[1], sA1)
        pop_g_task()
        emit_sub(3, pm[0], pm[1], sA1)
        emit_b(1, sA1, sBt)
        emit_pool_c(L)
        del hist[L]
        for fn, k in g_tasks:
            fn(k)
        g_tasks.clear()
        first_unpushed = ((nchunk - 5) // 4 + 1) if nchunk >= 5 else 0
        last_k = cpc // 512 - 1
        for k in range(first_unpushed, last_k + 1):
            g_task_g0(k); g_task_lo(k); g_task_hi(k)

    nc.compile()
    return nc


def _host_pack(points, W0, b0, W1, b1, W2, b2, G0, g0, G1, g1, n4c):
    """Build per-core input maps (host-side layout prep, numpy only)."""
    bf16 = _bf16()
    n = n4c * 4 * NCORES

    # pts4[3a+f, q] = points[4q+a, f]
    pts4 = np.ascontiguousarray(
        points[:n].reshape(-1, 4, 3).transpose(1, 2, 0).reshape(12, -1)
    ).astype(bf16)

    # W0 block-diagonal over 4 points: [12, 128]
    w0q = np.zeros((12, 128), np.float32)
    for a in range(4):
        w0q[3 * a:3 * a + 3, 32 * a:32 * a + 32] = W0
    # W1A/W1B: rows 32a+f; cols 64a'+g ; a' in {0,1} / {2,3}
    w1a = np.zeros((128, 128), np.float32)
    w1b = np.zeros((128, 128), np.float32)
    for a in range(2):
        w1a[32 * a:32 * a + 32, 64 * a:64 * a + 64] = W1
        w1b[32 * (a + 2):32 * (a + 2) + 32, 64 * a:64 * a + 64] = W1
    # W2 duplicated on both partition halves
    w2d = np.concatenate([W2, W2], axis=0)

    wbig = np.concatenate(
        [w1a, w1b, w2d, G0, G1[:, :128], G1[:, 128:]], axis=1)
    bbig = np.stack([np.tile(b0, 4), np.tile(b1, 2), b2, g0,
                     g1[:128], g1[128:]], axis=1)
    common = {
        "w0q": w0q.astype(bf16),
        "wbig": wbig.astype(bf16),
        "bbig": bbig.astype(np.float32),
    }
    in_maps = []
    for c in range(NCORES):
        m = dict(common)
        m["pts4"] = np.ascontiguousarray(pts4[:, c * n4c:(c + 1) * n4c])
        in_maps.append(m)
    return in_maps


def _numpy_fallback(points, cluster, num_clusters,
                    W0, b0, W1, b1, W2, b2, G0, g0, G1, g1):
    h = points.astype(np.float32)
    for W, b in ((W0, b0), (W1, b1), (W2, b2)):
        h = np.maximum(h @ W + b, 0.0)
    order = np.argsort(cluster, kind="stable")
    cs = cluster[order]
    hs = h[order]
    starts = np.searchsorted(cs, np.arange(num_clusters), side="left")
    counts = np.bincount(cs, minlength=num_clusters)
    safe_starts = np.minimum(starts, max(len(hs) - 1, 0))
    seg = np.maximum.reduceat(hs, safe_starts, axis=0)
    seg[counts == 0] = -np.inf   # match segment_max identity on empties
    pooled = seg
    gx = pooled
    for W, b in ((G0, g0), (G1, g1)):
        gx = np.maximum(gx @ W + b, 0.0)
    return gx.astype(np.float32)


def kernel(**inputs) -> np.ndarray:
    points = np.asarray(inputs["points"], np.float32)
    cluster = np.asarray(inputs["cluster"]).astype(np.int64)
    num_clusters = int(np.asarray(inputs["num_clusters"]))
    W0 = np.asarray(inputs["W0"], np.float32); b0 = np.asarray(inputs["b0"], np.float32)
    W1 = np.asarray(inputs["W1"], np.float32); b1 = np.asarray(inputs["b1"], np.float32)
    W2 = np.asarray(inputs["W2"], np.float32); b2 = np.asarray(inputs["b2"], np.float32)
    G0 = np.asarray(inputs["G0"], np.float32); g0 = np.asarray(inputs["g0"], np.float32)
    G1 = np.asarray(inputs["G1"], np.float32); g1 = np.asarray(inputs["g1"], np.float32)

    expected = (points.shape == (N, 3) and num_clusters == C
                and cluster.shape == (N,))
    if expected:
        # contiguous equal clusters of 32 points, as produced by setup_inputs
        expected = bool(
            np.array_equal(cluster[::PTS], np.arange(C, dtype=np.int64))
            and np.array_equal(cluster, np.repeat(cluster[::PTS], PTS))
        )
    if not expected:
        return _numpy_fallback(points, cluster, num_clusters,
                               W0, b0, W1, b1, W2, b2, G0, g0, G1, g1)

    from concourse.bass_utils import run_bass_kernel_spmd

    if "nc" not in _CACHE:
        _CACHE["nc"] = _build_module(N4C)
    nc = _CACHE["nc"]

    in_maps = _host_pack(points, W0, b0, W1, b1, W2, b2, G0, g0, G1, g1, N4C)
    res = run_bass_kernel_spmd(nc, in_maps, core_ids=list(range(NCORES)))
    outs = []
    for c in range(NCORES):
        o = np.asarray(res.results[c]["outt"]).astype(np.float32)  # [256, CPC]
        outs.append(o.T)                                           # [CPC, 256]
    return np.ascontiguousarray(np.concatenate(outs, axis=0))



# revision 19
# speedup vs baseline: 1.1457x; 1.1457x over previous
"""Trainium2 Bass kernel for a PointNet-style neighborhood encoder.

Computation (matches the reference nn.Module):
    h = relu(relu(relu(points @ W0 + b0) @ W1 + b1) @ W2 + b2)   # [N,3] -> [N,128]
    pooled = segment_max(h, cluster)                             # [C,128], 32 pts/cluster
    out = relu(relu(pooled @ G0 + g0) @ G1 + g1)                 # [C,256]

Sharding: data-parallel over points across 8 NeuronCores (cluster
boundaries are shard-aligned because clusters are contiguous, 32
points each). Weights are replicated. No collectives; the host
scatters inputs and gathers per-core outputs.

Device strategy (per core, n = 262144 points = 65536 quad-columns):
  - Host packs points feature-major, 4 points per 128-partition column
    ("quads"): pts4[3a+f, q] = points[4q+a, f], so layer 0 is a single
    block-diagonal matmul (K=12, M=128) producing h0 for 4 points/col.
  - Layer 1 uses two permuted block-diagonal stationaries W1A/W1B
    (K=128, M=128) producing h1 with 2 points per column.
  - Layer 2 uses W2 duplicated on both partition halves; 4 sub-matmuls
    (K=64, M=128) with rhs partition slices map to distinct PE row
    groups, producing z = W2^T h1 (bias/relu deferred) in PSUM.
  - segment_max: relu is monotone and b2 is constant per feature, so
    pooled = relu(max_p(z) + b2). max over (4 tensors x 8 quads) is ONE
    VectorE tensor_reduce(axis=XY) straight out of PSUM per sub-chunk.
  - ScalarE (ACT) does every relu+bias PSUM->SBUF evacuation; VectorE
    only does the pooling reduces. bf16 activations everywhere
    (PSUM stays f32 as the HW requires).
  - Global MLP on pooled [128, 8192] per core; output is written
    feature-major [256, 8192] bf16 and transposed/upcast on the host.
"""

import numpy as np

# ---- problem geometry (hardcoded per contract) ----
N = 2097152          # total points
C = 65536            # clusters
PTS = 32             # points per cluster
NCORES = 8
NPC = N // NCORES    # points per core = 262144
N4C = NPC // 4       # quad-columns per core = 65536
CPC = C // NCORES    # clusters per core = 8192

BIG = 1024           # quad-columns per big-chunk
SUB = 256            # quad-columns per L2/pool sub-chunk
NCHUNK = N4C // BIG  # 64
NSUB = BIG // SUB    # 4

_CACHE = {}


def _bf16():
    import ml_dtypes
    return ml_dtypes.bfloat16


def _build_module(n4c: int):
    """Build the Bass module (SPMD program, same for all cores)."""
    import concourse.bass as bass
    import concourse.bacc as bacc
    import concourse.tile as tile
    from concourse import mybir

    BF = mybir.dt.bfloat16
    F32 = mybir.dt.float32
    RELU = mybir.ActivationFunctionType.Relu
    MAX = mybir.AluOpType.max
    ADD = mybir.AluOpType.add
    AXX = mybir.AxisListType.X

    nchunk = n4c // BIG
    cpc = n4c // 8          # clusters per core for this size

    nc = bacc.Bacc()

    # ---- DRAM I/O ----
    # weights batched into two tensors so the prologue is 3 parallel DMAs
    # (13 small serialized DMAs used to block the Pool queue for ~6.5us)
    pts4 = nc.dram_tensor("pts4", [12, n4c], BF, kind="ExternalInput")
    w0q = nc.dram_tensor("w0q", [12, 128], BF, kind="ExternalInput")
    wbig = nc.dram_tensor("wbig", [128, 768], BF, kind="ExternalInput")
    bbig = nc.dram_tensor("bbig", [128, 6], F32, kind="ExternalInput")
    outt = nc.dram_tensor("outt", [256, cpc], BF, kind="ExternalOutput")

    from contextlib import ExitStack
    with tile.TileContext(nc) as tc, ExitStack() as ctx:
        singles = ctx.enter_context(tc.tile_pool(name="singles", bufs=1))
        ppts = ctx.enter_context(tc.tile_pool(name="ppts", bufs=3))
        ph0s = ctx.enter_context(tc.tile_pool(name="ph0s", bufs=2))
        ph1s = ctx.enter_context(tc.tile_pool(name="ph1s", bufs=3))
        psA = ctx.enter_context(tc.tile_pool(name="psA", bufs=3))
        psB = ctx.enter_context(tc.tile_pool(name="psB", bufs=2))
        psC = ctx.enter_context(tc.tile_pool(name="psC", bufs=2))
        psum_h1 = ctx.enter_context(tc.tile_pool(name="psum_h1", bufs=2, space="PSUM"))
        psum_z = ctx.enter_context(tc.tile_pool(name="psum_z", bufs=4, space="PSUM"))

        # ---- load constants: 3 batched DMAs on otherwise-idle queues ----
        wbig_s = singles.tile([128, 768], BF)
        nc.gpsimd.dma_start(out=wbig_s[:], in_=wbig[:])
        w0q_s = singles.tile([12, 128], BF)
        nc.scalar.dma_start(out=w0q_s[:], in_=w0q[:])
        bbig_s = singles.tile([128, 6], F32)
        nc.scalar.dma_start(out=bbig_s[:], in_=bbig[:])
        w1a_s = wbig_s[:, 0:128]
        w1b_s = wbig_s[:, 128:256]
        w2d_s = wbig_s[:, 256:384]
        g0w_s = wbig_s[:, 384:512]
        g1lo_s = wbig_s[:, 512:640]
        g1hi_s = wbig_s[:, 640:768]
        b0q_s = bbig_s[:, 0:1]
        b1d_s = bbig_s[:, 1:2]
        b2v_s = bbig_s[:, 2:3]
        g0v_s = bbig_s[:, 3:4]
        g1l_s = bbig_s[:, 4:5]
        g1h_s = bbig_s[:, 5:6]

        # pooled max(z) accumulator for the whole core
        pooled = singles.tile([128, cpc], BF)

        # ---- main loop; L2/pool of chunk i-2 runs during chunk i ----
        # The 2-chunk shift lets the L1 evacuations be two single
        # [128,1024] ACT instructions (cheapest form) and still complete
        # well before their L2 consumers.
        # Per chunk (1024 quad-cols = 4096 points) steady-state budget:
        #   PE   L0 2x512 + L1 4x512 + L2 16x256 + G mms      ~3150 ns
        #   ACT  2x h0 evac + 2x [128,1024] h1 evac           ~3300 ns
        #   DVE  4x A-TT (PSUM, the only engine that can) + G ~3130 ns
        #   Pool B/C tree stages + g0in (SBUF-only by HW rule) ~1710 ns
        # Max-pool tree (4096 z cols -> 128 pooled, max over 32 pts):
        #   A (per sub, DVE):   TT max(tileA, tileB) PSUM->bf16 SBUF
        #   B (per pair, Pool): TT max of a/b halves
        #   C1/C2/C3 (Pool):    TT chain folding quads 8->4->2->1
        # PSUM (8 banks): h0 ring 1x[128,512], h1 ring 2x[128,1024] (also
        # hosts the [128,512] G-MLP matmul tiles), z ring 3x[128,512].
        cur_sB = {}

        def emit_l2_pair(i, jpair, h1as, h1bs):
            """L2 matmuls for subs (2*jpair, 2*jpair+1) of chunk i: per sub
            two [128,512] z tiles (tileA = a0|b0, tileB = a1|b1), then the
            A-stage TT on DVE merges them into sAp."""
            if jpair == 0:
                sBt = psB.tile([128, BIG], BF, tag="sB")
                cur_sB[i] = sBt
            else:
                sBt = cur_sB[i]
            sAp = psA.tile([128, 4 * SUB], BF, tag="sA")
            for idx, j in ((0, 2 * jpair), (1, 2 * jpair + 1)):
                s0 = j * SUB
                zpa = psum_z.tile([128, 2 * SUB], F32, tag="zp")
                zpb = psum_z.tile([128, 2 * SUB], F32, tag="zp")
                nc.tensor.matmul(zpa[:, 0:SUB],
                                 w2d_s[0:64, :], h1as[0:64, s0:s0 + SUB])
                nc.tensor.matmul(zpa[:, SUB:2 * SUB],
                                 w2d_s[0:64, :], h1bs[0:64, s0:s0 + SUB])
                nc.tensor.matmul(zpb[:, 0:SUB],
                                 w2d_s[64:128, :], h1as[64:128, s0:s0 + SUB])
                nc.tensor.matmul(zpb[:, SUB:2 * SUB],
                                 w2d_s[64:128, :], h1bs[64:128, s0:s0 + SUB])
                # A: merge point-groups {a0,a1} and {b0,b1} (DVE; Pool
                # cannot read PSUM on real HW)
                nc.vector.tensor_tensor(
                    out=sAp[:, idx * 512:(idx + 1) * 512],
                    in0=zpa[:], in1=zpb[:], op=MAX)
            # B: merge a/b -> (sub, cluster, quad) layout, on Pool
            sAv = sAp.rearrange("p (s t x) -> p s t x", s=2, t=2)
            nc.gpsimd.tensor_tensor(
                out=sBt[:, jpair * 512:(jpair + 1) * 512],
                in0=sAv[:, :, 0:1, :], in1=sAv[:, :, 1:2, :], op=MAX)

        def emit_pool_c(i):
            # C: fold the 8 quads per cluster with a TT chain on Pool
            sBt = cur_sB.pop(i)
            base = i * (BIG // 8)
            sC1 = psC.tile([128, 512], BF, tag="sC1")
            sBv = sBt.rearrange("p (c q) -> p c q", q=8)
            nc.gpsimd.tensor_tensor(out=sC1[:], in0=sBv[:, :, 0:4],
                                    in1=sBv[:, :, 4:8], op=MAX)
            sC2 = psC.tile([128, 256], BF, tag="sC2")
            sC1v = sC1.rearrange("p (c q) -> p c q", q=4)
            nc.gpsimd.tensor_tensor(out=sC2[:], in0=sC1v[:, :, 0:2],
                                    in1=sC1v[:, :, 2:4], op=MAX)
            sC2v = sC2.rearrange("p (c q) -> p c q", q=2)
            nc.gpsimd.tensor_tensor(out=pooled[:, base:base + BIG // 8],
                                    in0=sC2v[:, :, 0:1], in1=sC2v[:, :, 1:2],
                                    op=MAX)

        g0in = singles.tile([128, cpc], BF)
        g1in = singles.tile([128, cpc], BF)
        goutL = singles.tile([128, cpc], BF)
        goutH = singles.tile([128, cpc], BF)

        # global-MLP tasks at 512-cluster granularity (one G matmul tile
        # [128,512] per task, allocated from the h1 PSUM ring; at most one
        # task per chunk so the ring rotation stays safe)
        def g_task_g0(k):
            sl = slice(k * 512, (k + 1) * 512)
            # relu(x + b) as (x add bias) max 0: g0in on Pool (SBUF only)
            nc.gpsimd.tensor_scalar(out=g0in[:, sl], in0=pooled[:, sl],
                                    scalar1=b2v_s[:], scalar2=0.0,
                                    op0=ADD, op1=MAX)
            gp = psum_z.tile([128, 512], F32, tag="zp")
            nc.tensor.matmul(gp[:], g0w_s[:], g0in[:, sl])
            nc.vector.tensor_scalar(out=g1in[:, sl], in0=gp[:],
                                    scalar1=g0v_s[:], scalar2=0.0,
                                    op0=ADD, op1=MAX)

        def g_task_lo(k):
            sl = slice(k * 512, (k + 1) * 512)
            gpl = psum_z.tile([128, 512], F32, tag="zp")
            nc.tensor.matmul(gpl[:], g1lo_s[:], g1in[:, sl])
            nc.vector.tensor_scalar(out=goutL[:, sl], in0=gpl[:],
                                    scalar1=g1l_s[:], scalar2=0.0,
                                    op0=ADD, op1=MAX)
            nc.sync.dma_start(out=outt[0:128, sl], in_=goutL[:, sl])

        def g_task_hi(k):
            sl = slice(k * 512, (k + 1) * 512)
            gph = psum_z.tile([128, 512], F32, tag="zp")
            nc.tensor.matmul(gph[:], g1hi_s[:], g1in[:, sl])
            nc.vector.tensor_scalar(out=goutH[:, sl], in0=gph[:],
                                    scalar1=g1h_s[:], scalar2=0.0,
                                    op0=ADD, op1=MAX)
            nc.sync.dma_start(out=outt[128:256, sl], in_=goutH[:, sl])

        g_tasks = []

        def pop_g_task():
            if g_tasks:
                fn, k = g_tasks.pop(0)
                fn(k)

        # Software-pipelined loop. Iteration `it` emits, interleaved:
        #   - L0 of chunk it        (so the h0 evacs precede the h1 evacs
        #                            of chunk it-1 in the ACT queue and no
        #                            ACT->PE->ACT serialization cycle forms)
        #   - L1 of chunk it-1
        #   - L2 + pooling of chunk it-2
        # L2 sub matmuls interleave with the L0/L1 matmuls so the PE covers
        # the A-stage latencies on the 3-slot z ring.
        hist = {}      # chunk -> (h1as, h1bs) awaiting their L2 stage
        h0_cur = None  # h0s of the chunk whose L1 stage runs next

        def emit_sub(j, h1as, h1bs, sAp):
            """One L2 sub (4 matmuls into two [128,512] z tiles) + A-TT."""
            s0 = j * SUB
            zpa = psum_z.tile([128, 2 * SUB], F32, tag="zp")
            zpb = psum_z.tile([128, 2 * SUB], F32, tag="zp")
            nc.tensor.matmul(zpa[:, 0:SUB],
                             w2d_s[0:64, :], h1as[0:64, s0:s0 + SUB])
            nc.tensor.matmul(zpa[:, SUB:2 * SUB],
                             w2d_s[0:64, :], h1bs[0:64, s0:s0 + SUB])
            nc.tensor.matmul(zpb[:, 0:SUB],
                             w2d_s[64:128, :], h1as[64:128, s0:s0 + SUB])
            nc.tensor.matmul(zpb[:, SUB:2 * SUB],
                             w2d_s[64:128, :], h1bs[64:128, s0:s0 + SUB])
            nc.vector.tensor_tensor(
                out=sAp[:, (j % 2) * 512:(j % 2 + 1) * 512],
                in0=zpa[:], in1=zpb[:], op=MAX)

        def emit_b(jpair, sAp, sBt):
            sAv = sAp.rearrange("p (s t x) -> p s t x", s=2, t=2)
            nc.gpsimd.tensor_tensor(
                out=sBt[:, jpair * 512:(jpair + 1) * 512],
                in0=sAv[:, :, 0:1, :], in1=sAv[:, :, 1:2, :], op=MAX)

        for it in range(nchunk + 1):
            do_l0 = it < nchunk
            ip = it - 2          # chunk whose L2+pool stage runs now
            prev_ready = ip >= 0

            if prev_ready:
                pm = hist[ip]
                sBt = psB.tile([128, BIG], BF, tag="sB")
                cur_sB[ip] = sBt
                sA0 = psA.tile([128, 4 * SUB], BF, tag="sA")
                sA1 = psA.tile([128, 4 * SUB], BF, tag="sA")

            if do_l0:
                c0 = it * BIG
                pts_t = ppts.tile([12, BIG], BF, tag="pts")
                nc.sync.dma_start(out=pts_t[:], in_=pts4[:, c0:c0 + BIG])
                h0s_n = ph0s.tile([128, BIG], BF, tag="h0s")
                h0qa = psum_z.tile([128, 512], F32, tag="zp")
                nc.tensor.matmul(h0qa[:], w0q_s[:], pts_t[:, 0:512])
                nc.scalar.activation(h0s_n[:, 0:512], h0qa[:], RELU,
                                     bias=b0q_s[:])

            if prev_ready:
                emit_sub(0, pm[0], pm[1], sA0)

            if do_l0:
                h0qb = psum_z.tile([128, 512], F32, tag="zp")
                nc.tensor.matmul(h0qb[:], w0q_s[:], pts_t[:, 512:1024])
                nc.scalar.activation(h0s_n[:, 512:1024], h0qb[:], RELU,
                                     bias=b0q_s[:])
            if it == 0:
                h0_cur = h0s_n
                continue
            h0s = h0_cur

            # L1A of chunk it-1, interleaved with L2 subs of chunk it-2
            h1as = ph1s.tile([128, BIG], BF, tag="h1as")
            h1bs = ph1s.tile([128, BIG], BF, tag="h1bs")
            h1pa = psum_h1.tile([128, BIG], F32, tag="h1p")
            nc.tensor.matmul(h1pa[:, 0:512], w1a_s[:], h0s[:, 0:512])
            nc.tensor.matmul(h1pa[:, 512:1024], w1a_s[:], h0s[:, 512:1024])
            nc.scalar.activation(h1as[:], h1pa[:], RELU, bias=b1d_s[:])

            if prev_ready:
                emit_sub(1, pm[0], pm[1], sA0)
                emit_b(0, sA0, sBt)

            # L1B of chunk it-1
            h1pb = psum_h1.tile([128, BIG], F32, tag="h1p")
            nc.tensor.matmul(h1pb[:, 0:512], w1b_s[:], h0s[:, 0:512])
            nc.tensor.matmul(h1pb[:, 512:1024], w1b_s[:], h0s[:, 512:1024])
            nc.scalar.activation(h1bs[:], h1pb[:], RELU, bias=b1d_s[:])

            if prev_ready:
                emit_sub(2, pm[0], pm[1], sA1)
            pop_g_task()
            if prev_ready:
                emit_sub(3, pm[0], pm[1], sA1)
                emit_b(1, sA1, sBt)
                emit_pool_c(ip)
                del hist[ip]

            hist[it - 1] = (h1as, h1bs)
            h0_cur = h0s_n if do_l0 else None

            # global-MLP work: chunk c pools during iteration c+2, so
            # block k (chunks 4k..4k+3) is ready from iteration 4k+5 on.
            if it >= 5 and (it - 5) % 4 == 0:
                k = (it - 5) // 4
                g_tasks.extend([(g_task_g0, k), (g_task_lo, k),
                                (g_task_hi, k)])

        # epilogue: the last chunk's L2+pool, then leftover G tasks
        L = nchunk - 1
        pm = hist[L]
        sBt = psB.tile([128, BIG], BF, tag="sB")
        cur_sB[L] = sBt
        sA0 = psA.tile([128, 4 * SUB], BF, tag="sA")
        sA1 = psA.tile([128, 4 * SUB], BF, tag="sA")
        emit_sub(0, pm[0], pm[1], sA0)
        pop_g_task()
        emit_sub(1, pm[0], pm[1], sA0)
        emit_b(0, sA0, sBt)
        pop_g_task()
        emit_sub(2, pm[0], pm[1], sA1)
        pop_g_task()
        emit_sub(3, pm[0], pm[1], sA1)
        emit_b(1, sA1, sBt)
        emit_pool_c(L)
        del hist[L]
        for fn, k in g_tasks:
            fn(k)
        g_tasks.clear()
        first_unpushed = ((nchunk - 5) // 4 + 1) if nchunk >= 5 else 0
        last_k = cpc // 512 - 1
        for k in range(first_unpushed, last_k + 1):
            g_task_g0(k); g_task_lo(k); g_task_hi(k)

    nc.compile()
    return nc


def _host_pack(points, W0, b0, W1, b1, W2, b2, G0, g0, G1, g1, n4c):
    """Build per-core input maps (host-side layout prep, numpy only)."""
    bf16 = _bf16()
    n = n4c * 4 * NCORES

    # pts4[3a+f, q] = points[4q+a, f]
    pts4 = np.ascontiguousarray(
        points[:n].reshape(-1, 4, 3).transpose(1, 2, 0).reshape(12, -1)
    ).astype(bf16)

    # W0 block-diagonal over 4 points: [12, 128]
    w0q = np.zeros((12, 128), np.float32)
    for a in range(4):
        w0q[3 * a:3 * a + 3, 32 * a:32 * a + 32] = W0
    # W1A/W1B: rows 32a+f; cols 64a'+g ; a' in {0,1} / {2,3}
    w1a = np.zeros((128, 128), np.float32)
    w1b = np.zeros((128, 128), np.float32)
    for a in range(2):
        w1a[32 * a:32 * a + 32, 64 * a:64 * a + 64] = W1
        w1b[32 * (a + 2):32 * (a + 2) + 32, 64 * a:64 * a + 64] = W1
    # W2 duplicated on both partition halves
    w2d = np.concatenate([W2, W2], axis=0)

    wbig = np.concatenate(
        [w1a, w1b, w2d, G0, G1[:, :128], G1[:, 128:]], axis=1)
    bbig = np.stack([np.tile(b0, 4), np.tile(b1, 2), b2, g0,
                     g1[:128], g1[128:]], axis=1)
    common = {
        "w0q": w0q.astype(bf16),
        "wbig": wbig.astype(bf16),
        "bbig": bbig.astype(np.float32),
    }
    in_maps = []
    for c in range(NCORES):
        m = dict(common)
        m["pts4"] = np.ascontiguousarray(pts4[:, c * n4c:(c + 1) * n4c])
        in_maps.append(m)
    return in_maps


def _numpy_fallback(points, cluster, num_clusters,
                    W0, b0, W1, b1, W2, b2, G0, g0, G1, g1):
    h = points.astype(np.float32)
    for W, b in ((W0, b0), (W1, b1), (W2, b2)):
        h = np.maximum(h @ W + b, 0.0)
    order = np.argsort(cluster, kind="stable")
    cs = cluster[order]
    hs = h[order]
    starts = np.searchsorted(cs, np.arange(num_clusters), side="left")
    counts = np.bincount(cs, minlength=num_clusters)
    safe_starts = np.minimum(starts, max(len(hs) - 1, 0))
    seg = np.maximum.reduceat(hs, safe_starts, axis=0)
    seg[counts == 0] = -np.inf   # match segment_max identity on empties
    pooled = seg
    gx = pooled
    for W, b in ((G0, g0), (G1, g1)):
        gx = np.maximum(gx @ W + b, 0.0)
    return gx.astype(np.float32)


def kernel(**inputs) -> np.ndarray:
    points = np.asarray(inputs["points"], np.float32)
    cluster = np.asarray(inputs["cluster"]).astype(np.int64)
    num_clusters = int(np.asarray(inputs["num_clusters"]))
    W0 = np.asarray(inputs["W0"], np.float32); b0 = np.asarray(inputs["b0"], np.float32)
    W1 = np.asarray(inputs["W1"], np.float32); b1 = np.asarray(inputs["b1"], np.float32)
    W2 = np.asarray(inputs["W2"], np.float32); b2 = np.asarray(inputs["b2"], np.float32)
    G0 = np.asarray(inputs["G0"], np.float32); g0 = np.asarray(inputs["g0"], np.float32)
    G1 = np.asarray(inputs["G1"], np.float32); g1 = np.asarray(inputs["g1"], np.float32)

    expected = (points.shape == (N, 3) and num_clusters == C
                and cluster.shape == (N,))
    if expected:
        # contiguous equal clusters of 32 points, as produced by setup_inputs
        expected = bool(
            np.array_equal(cluster[::PTS], np.arange(C, dtype=np.int64))
            and np.array_equal(cluster, np.repeat(cluster[::PTS], PTS))
        )
    if not expected:
        return _numpy_fallback(points, cluster, num_clusters,
                               W0, b0, W1, b1, W2, b2, G0, g0, G1, g1)

    from concourse.bass_utils import run_bass_kernel_spmd

    if "nc" not in _CACHE:
        _CACHE["nc"] = _build_module(N4C)
    nc = _CACHE["nc"]

    in_maps = _host_pack(points, W0, b0, W1, b1, W2, b2, G0, g0, G1, g1, N4C)
    res = run_bass_kernel_spmd(nc, in_maps, core_ids=list(range(NCORES)))
    outs = []
    for c in range(NCORES):
        o = np.asarray(res.results[c]["outt"]).astype(np.float32)  # [256, CPC]
        outs.append(o.T)                                           # [CPC, 256]
    return np.ascontiguousarray(np.concatenate(outs, axis=0))



# revision 20
# speedup vs baseline: 1.2912x; 1.1270x over previous
"""Trainium2 Bass kernel for a PointNet-style neighborhood encoder.

Computation (matches the reference nn.Module):
    h = relu(relu(relu(points @ W0 + b0) @ W1 + b1) @ W2 + b2)   # [N,3] -> [N,128]
    pooled = segment_max(h, cluster)                             # [C,128], 32 pts/cluster
    out = relu(relu(pooled @ G0 + g0) @ G1 + g1)                 # [C,256]

Sharding: data-parallel over points across 8 NeuronCores (cluster
boundaries are shard-aligned because clusters are contiguous, 32
points each). Weights are replicated. No collectives; the host
scatters inputs and gathers per-core outputs.

Device strategy (per core, n = 262144 points = 65536 quad-columns):
  - Host packs points feature-major, 4 points per 128-partition column
    ("quads"): pts4[3a+f, q] = points[4q+a, f], so layer 0 is a single
    block-diagonal matmul (K=12, M=128) producing h0 for 4 points/col.
  - Layer 1 uses two permuted block-diagonal stationaries W1A/W1B
    (K=128, M=128) producing h1 with 2 points per column.
  - Layer 2 uses W2 duplicated on both partition halves; 4 sub-matmuls
    (K=64, M=128) with rhs partition slices map to distinct PE row
    groups, producing z = W2^T h1 (bias/relu deferred) in PSUM.
  - segment_max: relu is monotone and b2 is constant per feature, so
    pooled = relu(max_p(z) + b2). max over (4 tensors x 8 quads) is ONE
    VectorE tensor_reduce(axis=XY) straight out of PSUM per sub-chunk.
  - ScalarE (ACT) does every relu+bias PSUM->SBUF evacuation; VectorE
    only does the pooling reduces. bf16 activations everywhere
    (PSUM stays f32 as the HW requires).
  - Global MLP on pooled [128, 8192] per core; output is written
    feature-major [256, 8192] bf16 and transposed/upcast on the host.
"""

import numpy as np

# ---- problem geometry (hardcoded per contract) ----
N = 2097152          # total points
C = 65536            # clusters
PTS = 32             # points per cluster
NCORES = 8
NPC = N // NCORES    # points per core = 262144
N4C = NPC // 4       # quad-columns per core = 65536
CPC = C // NCORES    # clusters per core = 8192

BIG = 1024           # quad-columns per big-chunk
SUB = 256            # quad-columns per L2/pool sub-chunk
NCHUNK = N4C // BIG  # 64
NSUB = BIG // SUB    # 4

_CACHE = {}


def _bf16():
    import ml_dtypes
    return ml_dtypes.bfloat16


def _build_module(n4c: int):
    """Build the Bass module (SPMD program, same for all cores)."""
    import concourse.bass as bass
    import concourse.bacc as bacc
    import concourse.tile as tile
    from concourse import mybir

    BF = mybir.dt.bfloat16
    F32 = mybir.dt.float32
    RELU = mybir.ActivationFunctionType.Relu
    MAX = mybir.AluOpType.max
    ADD = mybir.AluOpType.add
    AXX = mybir.AxisListType.X

    nchunk = n4c // BIG
    cpc = n4c // 8          # clusters per core for this size

    nc = bacc.Bacc()

    # ---- DRAM I/O ----
    # weights batched into two tensors so the prologue is 3 parallel DMAs
    # (13 small serialized DMAs used to block the Pool queue for ~6.5us)
    pts4 = nc.dram_tensor("pts4", [12, n4c], BF, kind="ExternalInput")
    w0q = nc.dram_tensor("w0q", [12, 128], BF, kind="ExternalInput")
    wbig = nc.dram_tensor("wbig", [128, 768], BF, kind="ExternalInput")
    bbig = nc.dram_tensor("bbig", [128, 6], F32, kind="ExternalInput")
    outt = nc.dram_tensor("outt", [256, cpc], BF, kind="ExternalOutput")

    from contextlib import ExitStack
    with tile.TileContext(nc) as tc, ExitStack() as ctx:
        singles = ctx.enter_context(tc.tile_pool(name="singles", bufs=1))
        ppts = ctx.enter_context(tc.tile_pool(name="ppts", bufs=3))
        ph0s = ctx.enter_context(tc.tile_pool(name="ph0s", bufs=2))
        ph1s = ctx.enter_context(tc.tile_pool(name="ph1s", bufs=3))
        psA = ctx.enter_context(tc.tile_pool(name="psA", bufs=3))
        psB = ctx.enter_context(tc.tile_pool(name="psB", bufs=2))
        psC = ctx.enter_context(tc.tile_pool(name="psC", bufs=2))
        psum_h1 = ctx.enter_context(tc.tile_pool(name="psum_h1", bufs=2, space="PSUM"))
        psum_z = ctx.enter_context(tc.tile_pool(name="psum_z", bufs=4, space="PSUM"))

        # ---- load constants: 3 batched DMAs on otherwise-idle queues ----
        wbig_s = singles.tile([128, 768], BF)
        nc.gpsimd.dma_start(out=wbig_s[:], in_=wbig[:])
        w0q_s = singles.tile([12, 128], BF)
        nc.scalar.dma_start(out=w0q_s[:], in_=w0q[:])
        bbig_s = singles.tile([128, 6], F32)
        nc.scalar.dma_start(out=bbig_s[:], in_=bbig[:])
        w1a_s = wbig_s[:, 0:128]
        w1b_s = wbig_s[:, 128:256]
        w2d_s = wbig_s[:, 256:384]
        g0w_s = wbig_s[:, 384:512]
        g1lo_s = wbig_s[:, 512:640]
        g1hi_s = wbig_s[:, 640:768]
        b0q_s = bbig_s[:, 0:1]
        b1d_s = bbig_s[:, 1:2]
        b2v_s = bbig_s[:, 2:3]
        g0v_s = bbig_s[:, 3:4]
        g1l_s = bbig_s[:, 4:5]
        g1h_s = bbig_s[:, 5:6]

        # pooled max(z) accumulator for the whole core
        pooled = singles.tile([128, cpc], BF)

        # ---- main loop; L2/pool of chunk i-2 runs during chunk i ----
        # The 2-chunk shift lets the L1 evacuations be two single
        # [128,1024] ACT instructions (cheapest form) and still complete
        # well before their L2 consumers.
        # Per chunk (1024 quad-cols = 4096 points) steady-state budget:
        #   PE   L0 2x512 + L1 4x512 + L2 16x256 + G mms      ~3150 ns
        #   ACT  2x h0 evac + 2x [128,1024] h1 evac           ~3300 ns
        #   DVE  4x A-TT (PSUM, the only engine that can) + G ~3130 ns
        #   Pool B/C tree stages + g0in (SBUF-only by HW rule) ~1710 ns
        # Max-pool tree (4096 z cols -> 128 pooled, max over 32 pts):
        #   A (per sub, DVE):   TT max(tileA, tileB) PSUM->bf16 SBUF
        #   B (per pair, Pool): TT max of a/b halves
        #   C1/C2/C3 (Pool):    TT chain folding quads 8->4->2->1
        # PSUM (8 banks): h0 ring 1x[128,512], h1 ring 2x[128,1024] (also
        # hosts the [128,512] G-MLP matmul tiles), z ring 3x[128,512].
        cur_sB = {}

        def emit_l2_pair(i, jpair, h1as, h1bs):
            """L2 matmuls for subs (2*jpair, 2*jpair+1) of chunk i: per sub
            two [128,512] z tiles (tileA = a0|b0, tileB = a1|b1), then the
            A-stage TT on DVE merges them into sAp."""
            if jpair == 0:
                sBt = psB.tile([128, BIG], BF, tag="sB")
                cur_sB[i] = sBt
            else:
                sBt = cur_sB[i]
            sAp = psA.tile([128, 4 * SUB], BF, tag="sA")
            for idx, j in ((0, 2 * jpair), (1, 2 * jpair + 1)):
                s0 = j * SUB
                zpa = psum_z.tile([128, 2 * SUB], F32, tag="zp")
                zpb = psum_z.tile([128, 2 * SUB], F32, tag="zp")
                nc.tensor.matmul(zpa[:, 0:SUB],
                                 w2d_s[0:64, :], h1as[0:64, s0:s0 + SUB])
                nc.tensor.matmul(zpa[:, SUB:2 * SUB],
                                 w2d_s[0:64, :], h1bs[0:64, s0:s0 + SUB])
                nc.tensor.matmul(zpb[:, 0:SUB],
                                 w2d_s[64:128, :], h1as[64:128, s0:s0 + SUB])
                nc.tensor.matmul(zpb[:, SUB:2 * SUB],
                                 w2d_s[64:128, :], h1bs[64:128, s0:s0 + SUB])
                # A: merge point-groups {a0,a1} and {b0,b1} (DVE; Pool
                # cannot read PSUM on real HW)
                nc.vector.tensor_tensor(
                    out=sAp[:, idx * 512:(idx + 1) * 512],
                    in0=zpa[:], in1=zpb[:], op=MAX)
            # B: merge a/b -> (sub, cluster, quad) layout, on Pool
            sAv = sAp.rearrange("p (s t x) -> p s t x", s=2, t=2)
            nc.gpsimd.tensor_tensor(
                out=sBt[:, jpair * 512:(jpair + 1) * 512],
                in0=sAv[:, :, 0:1, :], in1=sAv[:, :, 1:2, :], op=MAX)

        def emit_pool_c(i):
            # C: fold the 8 quads per cluster with a TT chain on Pool
            sBt = cur_sB.pop(i)
            base = i * (BIG // 8)
            sC1 = psC.tile([128, 512], BF, tag="sC1")
            sBv = sBt.rearrange("p (c q) -> p c q", q=8)
            nc.gpsimd.tensor_tensor(out=sC1[:], in0=sBv[:, :, 0:4],
                                    in1=sBv[:, :, 4:8], op=MAX)
            sC2 = psC.tile([128, 256], BF, tag="sC2")
            sC1v = sC1.rearrange("p (c q) -> p c q", q=4)
            nc.gpsimd.tensor_tensor(out=sC2[:], in0=sC1v[:, :, 0:2],
                                    in1=sC1v[:, :, 2:4], op=MAX)
            sC2v = sC2.rearrange("p (c q) -> p c q", q=2)
            nc.gpsimd.tensor_tensor(out=pooled[:, base:base + BIG // 8],
                                    in0=sC2v[:, :, 0:1], in1=sC2v[:, :, 1:2],
                                    op=MAX)

        g0in = singles.tile([128, cpc], BF)
        g1in = singles.tile([128, cpc], BF)
        goutL = singles.tile([128, cpc], BF)
        goutH = singles.tile([128, cpc], BF)

        # global-MLP tasks at 512-cluster granularity (one G matmul tile
        # [128,512] per task, allocated from the h1 PSUM ring; at most one
        # task per chunk so the ring rotation stays safe)
        def g_task_g0(k):
            sl = slice(k * 512, (k + 1) * 512)
            # relu(x + b) as (x add bias) max 0: g0in on Pool (SBUF only)
            nc.gpsimd.tensor_scalar(out=g0in[:, sl], in0=pooled[:, sl],
                                    scalar1=b2v_s[:], scalar2=0.0,
                                    op0=ADD, op1=MAX)
            gp = psum_z.tile([128, 512], F32, tag="zp")
            nc.tensor.matmul(gp[:], g0w_s[:], g0in[:, sl])
            nc.vector.tensor_scalar(out=g1in[:, sl], in0=gp[:],
                                    scalar1=g0v_s[:], scalar2=0.0,
                                    op0=ADD, op1=MAX)

        def g_task_lo(k):
            sl = slice(k * 512, (k + 1) * 512)
            gpl = psum_z.tile([128, 512], F32, tag="zp")
            nc.tensor.matmul(gpl[:], g1lo_s[:], g1in[:, sl])
            nc.vector.tensor_scalar(out=goutL[:, sl], in0=gpl[:],
                                    scalar1=g1l_s[:], scalar2=0.0,
                                    op0=ADD, op1=MAX)
            nc.sync.dma_start(out=outt[0:128, sl], in_=goutL[:, sl])

        def g_task_hi(k):
            sl = slice(k * 512, (k + 1) * 512)
            gph = psum_z.tile([128, 512], F32, tag="zp")
            nc.tensor.matmul(gph[:], g1hi_s[:], g1in[:, sl])
            nc.vector.tensor_scalar(out=goutH[:, sl], in0=gph[:],
                                    scalar1=g1h_s[:], scalar2=0.0,
                                    op0=ADD, op1=MAX)
            nc.sync.dma_start(out=outt[128:256, sl], in_=goutH[:, sl])

        g_tasks = []

        def pop_g_task():
            if g_tasks:
                fn, k = g_tasks.pop(0)
                fn(k)

        # Software-pipelined loop. Iteration `it` emits, interleaved:
        #   - L0 of chunk it        (so the h0 evacs precede the h1 evacs
        #                            of chunk it-1 in the ACT queue and no
        #                            ACT->PE->ACT serialization cycle forms)
        #   - L1 of chunk it-1
        #   - L2 + pooling of chunk it-2
        # L2 sub matmuls interleave with the L0/L1 matmuls so the PE covers
        # the A-stage latencies on the 3-slot z ring.
        hist = {}      # chunk -> (h1as, h1bs) awaiting their L2 stage
        h0_cur = None  # h0s of the chunk whose L1 stage runs next

        def emit_sub(j, h1as, h1bs, sAp):
            """One L2 sub (4 matmuls into two [128,512] z tiles) + A-TT."""
            s0 = j * SUB
            zpa = psum_z.tile([128, 2 * SUB], F32, tag="zp")
            zpb = psum_z.tile([128, 2 * SUB], F32, tag="zp")
            nc.tensor.matmul(zpa[:, 0:SUB],
                             w2d_s[0:64, :], h1as[0:64, s0:s0 + SUB])
            nc.tensor.matmul(zpa[:, SUB:2 * SUB],
                             w2d_s[0:64, :], h1bs[0:64, s0:s0 + SUB])
            nc.tensor.matmul(zpb[:, 0:SUB],
                             w2d_s[64:128, :], h1as[64:128, s0:s0 + SUB])
            nc.tensor.matmul(zpb[:, SUB:2 * SUB],
                             w2d_s[64:128, :], h1bs[64:128, s0:s0 + SUB])
            nc.vector.tensor_tensor(
                out=sAp[:, (j % 2) * 512:(j % 2 + 1) * 512],
                in0=zpa[:], in1=zpb[:], op=MAX)

        def emit_b(jpair, sAp, sBt):
            sAv = sAp.rearrange("p (s t x) -> p s t x", s=2, t=2)
            nc.gpsimd.tensor_tensor(
                out=sBt[:, jpair * 512:(jpair + 1) * 512],
                in0=sAv[:, :, 0:1, :], in1=sAv[:, :, 1:2, :], op=MAX)

        for it in range(nchunk + 1):
            do_l0 = it < nchunk
            ip = it - 2          # chunk whose L2+pool stage runs now
            prev_ready = ip >= 0

            if prev_ready:
                pm = hist[ip]
                sBt = psB.tile([128, BIG], BF, tag="sB")
                cur_sB[ip] = sBt
                sA0 = psA.tile([128, 4 * SUB], BF, tag="sA")
                sA1 = psA.tile([128, 4 * SUB], BF, tag="sA")

            if do_l0:
                c0 = it * BIG
                pts_t = ppts.tile([12, BIG], BF, tag="pts")
                nc.sync.dma_start(out=pts_t[:], in_=pts4[:, c0:c0 + BIG])
                h0s_n = ph0s.tile([128, BIG], BF, tag="h0s")
                h0qa = psum_h1.tile([128, 512], F32, tag="h1p")
                nc.tensor.matmul(h0qa[:], w0q_s[:], pts_t[:, 0:512])
                nc.scalar.activation(h0s_n[:, 0:512], h0qa[:], RELU,
                                     bias=b0q_s[:])

            if prev_ready:
                emit_sub(0, pm[0], pm[1], sA0)

            if do_l0:
                h0qb = psum_h1.tile([128, 512], F32, tag="h1p")
                nc.tensor.matmul(h0qb[:], w0q_s[:], pts_t[:, 512:1024])
                nc.scalar.activation(h0s_n[:, 512:1024], h0qb[:], RELU,
                                     bias=b0q_s[:])
            if it == 0:
                h0_cur = h0s_n
                continue
            h0s = h0_cur

            # L1A of chunk it-1, interleaved with L2 subs of chunk it-2
            h1as = ph1s.tile([128, BIG], BF, tag="h1as")
            h1bs = ph1s.tile([128, BIG], BF, tag="h1bs")
            h1pa = psum_h1.tile([128, BIG], F32, tag="h1p")
            nc.tensor.matmul(h1pa[:, 0:512], w1a_s[:], h0s[:, 0:512])
            nc.tensor.matmul(h1pa[:, 512:1024], w1a_s[:], h0s[:, 512:1024])
            nc.scalar.activation(h1as[:], h1pa[:], RELU, bias=b1d_s[:])

            if prev_ready:
                emit_sub(1, pm[0], pm[1], sA0)
                emit_b(0, sA0, sBt)

            # L1B of chunk it-1
            h1pb = psum_h1.tile([128, BIG], F32, tag="h1p")
            nc.tensor.matmul(h1pb[:, 0:512], w1b_s[:], h0s[:, 0:512])
            nc.tensor.matmul(h1pb[:, 512:1024], w1b_s[:], h0s[:, 512:1024])
            nc.scalar.activation(h1bs[:], h1pb[:], RELU, bias=b1d_s[:])

            if prev_ready:
                emit_sub(2, pm[0], pm[1], sA1)
                emit_sub(3, pm[0], pm[1], sA1)
                emit_b(1, sA1, sBt)
                emit_pool_c(ip)
                del hist[ip]
            pop_g_task()

            hist[it - 1] = (h1as, h1bs)
            h0_cur = h0s_n if do_l0 else None

            # global-MLP work: chunk c pools during iteration c+2, so
            # block k (chunks 4k..4k+3) is ready from iteration 4k+5 on.
            if it >= 5 and (it - 5) % 4 == 0:
                k = (it - 5) // 4
                g_tasks.extend([(g_task_g0, k), (g_task_lo, k),
                                (g_task_hi, k)])

        # epilogue: the last chunk's L2+pool, then leftover G tasks
        L = nchunk - 1
        pm = hist[L]
        sBt = psB.tile([128, BIG], BF, tag="sB")
        cur_sB[L] = sBt
        sA0 = psA.tile([128, 4 * SUB], BF, tag="sA")
        sA1 = psA.tile([128, 4 * SUB], BF, tag="sA")
        emit_sub(0, pm[0], pm[1], sA0)
        pop_g_task()
        emit_sub(1, pm[0], pm[1], sA0)
        emit_b(0, sA0, sBt)
        pop_g_task()
        emit_sub(2, pm[0], pm[1], sA1)
        pop_g_task()
        emit_sub(3, pm[0], pm[1], sA1)
        emit_b(1, sA1, sBt)
        emit_pool_c(L)
        del hist[L]
        for fn, k in g_tasks:
            fn(k)
        g_tasks.clear()
        first_unpushed = ((nchunk - 5) // 4 + 1) if nchunk >= 5 else 0
        last_k = cpc // 512 - 1
        for k in range(first_unpushed, last_k + 1):
            g_task_g0(k); g_task_lo(k); g_task_hi(k)

    nc.compile()
    return nc


def _host_pack(points, W0, b0, W1, b1, W2, b2, G0, g0, G1, g1, n4c):
    """Build per-core input maps (host-side layout prep, numpy only)."""
    bf16 = _bf16()
    n = n4c * 4 * NCORES

    # pts4[3a+f, q] = points[4q+a, f]
    pts4 = np.ascontiguousarray(
        points[:n].reshape(-1, 4, 3).transpose(1, 2, 0).reshape(12, -1)
    ).astype(bf16)

    # W0 block-diagonal over 4 points: [12, 128]
    w0q = np.zeros((12, 128), np.float32)
    for a in range(4):
        w0q[3 * a:3 * a + 3, 32 * a:32 * a + 32] = W0
    # W1A/W1B: rows 32a+f; cols 64a'+g ; a' in {0,1} / {2,3}
    w1a = np.zeros((128, 128), np.float32)
    w1b = np.zeros((128, 128), np.float32)
    for a in range(2):
        w1a[32 * a:32 * a + 32, 64 * a:64 * a + 64] = W1
        w1b[32 * (a + 2):32 * (a + 2) + 32, 64 * a:64 * a + 64] = W1
    # W2 duplicated on both partition halves
    w2d = np.concatenate([W2, W2], axis=0)

    wbig = np.concatenate(
        [w1a, w1b, w2d, G0, G1[:, :128], G1[:, 128:]], axis=1)
    bbig = np.stack([np.tile(b0, 4), np.tile(b1, 2), b2, g0,
                     g1[:128], g1[128:]], axis=1)
    common = {
        "w0q": w0q.astype(bf16),
        "wbig": wbig.astype(bf16),
        "bbig": bbig.astype(np.float32),
    }
    in_maps = []
    for c in range(NCORES):
        m = dict(common)
        m["pts4"] = np.ascontiguousarray(pts4[:, c * n4c:(c + 1) * n4c])
        in_maps.append(m)
    return in_maps


def _numpy_fallback(points, cluster, num_clusters,
                    W0, b0, W1, b1, W2, b2, G0, g0, G1, g1):
    h = points.astype(np.float32)
    for W, b in ((W0, b0), (W1, b1), (W2, b2)):
        h = np.maximum(h @ W + b, 0.0)
    order = np.argsort(cluster, kind="stable")
    cs = cluster[order]
    hs = h[order]
    starts = np.searchsorted(cs, np.arange(num_clusters), side="left")
    counts = np.bincount(cs, minlength=num_clusters)
    safe_starts = np.minimum(starts, max(len(hs) - 1, 0))
    seg = np.maximum.reduceat(hs, safe_starts, axis=0)
    seg[counts == 0] = -np.inf   # match segment_max identity on empties
    pooled = seg
    gx = pooled
    for W, b in ((G0, g0), (G1, g1)):
        gx = np.maximum(gx @ W + b, 0.0)
    return gx.astype(np.float32)


def kernel(**inputs) -> np.ndarray:
    points = np.asarray(inputs["points"], np.float32)
    cluster = np.asarray(inputs["cluster"]).astype(np.int64)
    num_clusters = int(np.asarray(inputs["num_clusters"]))
    W0 = np.asarray(inputs["W0"], np.float32); b0 = np.asarray(inputs["b0"], np.float32)
    W1 = np.asarray(inputs["W1"], np.float32); b1 = np.asarray(inputs["b1"], np.float32)
    W2 = np.asarray(inputs["W2"], np.float32); b2 = np.asarray(inputs["b2"], np.float32)
    G0 = np.asarray(inputs["G0"], np.float32); g0 = np.asarray(inputs["g0"], np.float32)
    G1 = np.asarray(inputs["G1"], np.float32); g1 = np.asarray(inputs["g1"], np.float32)

    expected = (points.shape == (N, 3) and num_clusters == C
                and cluster.shape == (N,))
    if expected:
        # contiguous equal clusters of 32 points, as produced by setup_inputs
        expected = bool(
            np.array_equal(cluster[::PTS], np.arange(C, dtype=np.int64))
            and np.array_equal(cluster, np.repeat(cluster[::PTS], PTS))
        )
    if not expected:
        return _numpy_fallback(points, cluster, num_clusters,
                               W0, b0, W1, b1, W2, b2, G0, g0, G1, g1)

    from concourse.bass_utils import run_bass_kernel_spmd

    if "nc" not in _CACHE:
        _CACHE["nc"] = _build_module(N4C)
    nc = _CACHE["nc"]

    in_maps = _host_pack(points, W0, b0, W1, b1, W2, b2, G0, g0, G1, g1, N4C)
    res = run_bass_kernel_spmd(nc, in_maps, core_ids=list(range(NCORES)))
    outs = []
    for c in range(NCORES):
        o = np.asarray(res.results[c]["outt"]).astype(np.float32)  # [256, CPC]
        outs.append(o.T)                                           # [CPC, 256]
    return np.ascontiguousarray(np.concatenate(outs, axis=0))

